# revision 37
# baseline (speedup 1.0000x reference)
"""Multi-head attention with relative-position-bias MLP on 8 TRN2 NeuronCores.

Strategy: data-parallel over batch (B=8 -> 1 element per core, no
collectives). Host prep is layout/dtype only (bf16 casts, transposes, and
exp() of the tiny 63x63 rel-pos-bias table = ~7 MFLOP of a 66 GFLOP problem).

Design:
  - everything bf16 on SBUF (f32r matmul rate is the same; halves DMA+SBUF).
  - half-width (512-col) substages, c-half as the outer loop per head pair:
    every psum tile is a single bank -> 8 banks split as scores/qk/proj
    ring x4, AV accumulators x2, V x1, transpose staging x1, giving enough
    ring lookahead to keep PE continuously busy at max p-state.
  - natural-layout AV: out[nq, 65] matmuls with lhsT = P tile slices (half
    the AV columns of the transposed form); rhs is [v_h | ones] so column
    64 is the softmax denominator per query row. Chains are emitted
    group-major (interleaved psum accumulation groups are broken on this
    toolchain) by buffering a phase's 16 P tiles and riding the chains +
    finalize into the next phase's stages.
  - finalize: batched reciprocal (partition-aligned), per-partition
    tensor_scalar normalize to bf16, PE transpose rebuilds [c, n] for proj.
  - proj computed transposed (final^T[o,n]) so proj_b is a per-partition
    Act bias; out stored [C,N] bf16, transposed/upcast on host. The c=0
    proj half is interleaved into the last pair's stages.
  - inputs arrive as one multi-dim-AP HWDGE transfer per tensor on the SP
    queue (DMA issue holds the issuing sequencer ~0.65us, so few, large
    issues with no data-dependent waits); per-head bias tables are built
    by a single 3D-AP replicating DMA, fetched one pair ahead.
  - QKV for pair j+1 and V for pair j ride pair j's attention stages
    through the shared psum ring.
"""
import sys

import numpy as np

sys.path.insert(0, "/opt/trn_rl_repo")

import concourse.bass as bass  # noqa: E402
import concourse.mybir as mybir  # noqa: E402
import concourse.tile as tile  # noqa: E402
from concourse import bacc  # noqa: E402
from concourse.bass_utils import run_bass_kernel_spmd  # noqa: E402
from concourse.masks import make_identity  # noqa: E402

F32 = mybir.dt.float32
BF16 = mybir.dt.bfloat16
EXP = mybir.ActivationFunctionType.Exp
MULT = mybir.AluOpType.mult

B, N, C, H, D = 8, 1024, 768, 12, 64
SCALE = float(D) ** -0.5
NT = N // 128   # 8 token tiles
CT = C // 128   # 6 channel tiles
NP = H // 2     # 6 head pairs
TBLW = 3781     # replicated-table width (2016-wide views at 252*t stay in range)
TW = 4001       # DRAM table stride per head (>= 220 + TBLW, zero padded)


def _build_graph():
    nc = bacc.Bacc("TRN2", target_bir_lowering=False, debug=False,
                   enable_asserts=False, num_devices=B)
    xT_d = nc.dram_tensor("xT", [C, N], BF16, kind="ExternalInput")
    xRT_d = nc.dram_tensor("xRT", [C, N], BF16, kind="ExternalInput")
    wqkv_d = nc.dram_tensor("qkv_wT", [C, 3 * C], BF16, kind="ExternalInput")
    wproj_d = nc.dram_tensor("proj_wT", [C, C], BF16, kind="ExternalInput")
    pbc_d = nc.dram_tensor("proj_b_col", [128, CT], F32, kind="ExternalInput")
    tbl_d = nc.dram_tensor("rpb_tbl", [H, TW], BF16, kind="ExternalInput")
    out_d = nc.dram_tensor("out", [C, N], BF16, kind="ExternalOutput")

    with tile.TileContext(nc) as tc:
        _kern(tc, nc, xT_d, xRT_d, wqkv_d, wproj_d, pbc_d, tbl_d, out_d)
    nc.compile()
    return nc


def _kern(tc, nc, xT_d, xRT_d, wqkv_d, wproj_d, pbc_d, tbl_d, out_d):
    from contextlib import ExitStack

    with ExitStack() as es:
        persist = es.enter_context(tc.tile_pool(name="persist", bufs=1))
        ld = es.enter_context(tc.tile_pool(name="ld", bufs=1))
        tblp = es.enter_context(tc.tile_pool(name="tblp", bufs=6))
        eep = es.enter_context(tc.tile_pool(name="eep", bufs=8))
        ppp = es.enter_context(tc.tile_pool(name="ppp", bufs=24))
        finp = es.enter_context(tc.tile_pool(name="finp", bufs=2))
        tinp = es.enter_context(tc.tile_pool(name="tinp", bufs=12))
        fsbp = es.enter_context(tc.tile_pool(name="fsbp", bufs=1))
        # psum banks: scores/qk/proj ring 4 + av 2 + v 1 + tr 1
        mix = es.enter_context(tc.tile_pool(name="mix", bufs=4, space="PSUM"))
        avp = es.enter_context(tc.tile_pool(name="avp", bufs=2, space="PSUM"))
        vps = es.enter_context(tc.tile_pool(name="vps", bufs=1, space="PSUM"))
        trp = es.enter_context(tc.tile_pool(name="trp", bufs=1, space="PSUM"))

        # ---- persistent SBUF ----
        qk_sb = [persist.tile([128, N], BF16, tag=f"qk{i}", name=f"qk{i}")
                 for i in range(12)]   # 0..5 qT per pair, 6..11 kT per pair
        # per head 65 cols: [v(64) | ones(1)]; natural-layout AV gives
        # out[nq, 65] with col 64 = softmax denominator per query row
        vaug = [persist.tile([128, H * 65], BF16, tag=f"va{i}",
                             name=f"va{i}") for i in range(NT)]
        ident = persist.tile([128, 128], BF16, tag="ident")
        make_identity(nc, ident[:])
        warm = persist.tile([1, 1], F32, tag="warm")
        nc.vector.memset(warm[:], 0.0)
        nc.scalar.activation(warm[:], warm[:], EXP)
        outT = [persist.tile([128, N], BF16, tag=f"ot{i}", name=f"ot{i}")
                for i in range(NP)]
        pbc = persist.tile([128, CT], F32, tag="pbc")
        for t in range(NT):
            nc.gpsimd.memset(vaug[t][:], 1.0)

        # ---- input DMAs: one multi-dim-AP transfer per tensor ----
        # c-chunk-major combined tiles; all issued via SP HWDGE, no waits.
        xTt = ld.tile([128, CT * N], BF16, tag="xT")
        xRTt = ld.tile([128, CT * N], BF16, tag="xRT")
        w0q = ld.tile([128, CT * 128], BF16, tag="w0q")   # pair-0 q slices
        w0k = ld.tile([128, CT * 128], BF16, tag="w0k")
        w0v = ld.tile([128, CT * 128], BF16, tag="w0v")
        wqkvt = ld.tile([128, CT * 3 * C], BF16, tag="wqkv")
        pwTt = ld.tile([128, CT * C], BF16, tag="pwT")
        xT = [xTt[:, i * N:(i + 1) * N] for i in range(CT)]
        xRT = [xRTt[:, i * N:(i + 1) * N] for i in range(CT)]
        wqkv = [wqkvt[:, i * 3 * C:(i + 1) * 3 * C] for i in range(CT)]
        pwT = [pwTt[:, i * C:(i + 1) * C] for i in range(CT)]
        def _w0dma(sec, wt):
            nc.sync.dma_start(
                wt[:], bass.AP(wqkv_d, sec * C,
                               [[3 * C, 128], [128 * 3 * C, CT], [1, 128]]))

        nc.sync.dma_start(
            xTt[:, 0:3 * N],
            bass.AP(xT_d, 0, [[N, 128], [128 * N, 3], [1, N]]))
        _w0dma(0, w0q)
        nc.sync.dma_start(
            xTt[:, 3 * N:], bass.AP(xT_d, 3 * 128 * N,
                                    [[N, 128], [128 * N, 3], [1, N]]))
        nc.sync.dma_start(
            xRTt[:], bass.AP(xRT_d, 0, [[N, 128], [128 * N, CT], [1, N]]))
        _w0dma(1, w0k)
        _w0dma(2, w0v)
        wq0 = [w0q[:, i * 128:(i + 1) * 128] for i in range(CT)]
        wk0 = [w0k[:, i * 128:(i + 1) * 128] for i in range(CT)]
        wv0 = [w0v[:, i * 128:(i + 1) * 128] for i in range(CT)]
        # tables: one 3D-AP DMA per head, fetched one pair ahead
        tbl_tiles = {}

        def fetch_tbl_pair(j):
            for h in (2 * j, 2 * j + 1):
                t = tblp.tile([128, TBLW], BF16, tag="tbl", name=f"tbl{h}")
                nc.sync.dma_start(
                    t[:], bass.AP(tbl_d, h * TW,
                                  [[63, 4], [1, 32], [1, TBLW]]))
                tbl_tiles[h] = t

        fetch_tbl_pair(0)
        nc.sync.dma_start(
            wqkvt[:], bass.AP(wqkv_d, 0,
                              [[3 * C, 128], [128 * 3 * C, CT], [1, 3 * C]]))
        nc.sync.dma_start(
            pwTt[:], bass.AP(wproj_d, 0, [[C, 128], [128 * C, CT], [1, C]]))
        nc.sync.dma_start(pbc[:], pbc_d.ap()[:, :])

        # ---- qkv unit emitters ----
        def qk_unit(j, is_k):
            """Compute qT (or kT) for pair j into qk_sb[j (+6)]."""
            dst = qk_sb[6 + j] if is_k else qk_sb[j]
            rhs_src = xRT if is_k else xT
            for c in range(2):
                ps = mix.tile([128, 512], F32, tag="mx", name=f"qk{j}{is_k}{c}")
                for kt in range(CT):
                    if j == 0:
                        w = (wk0 if is_k else wq0)[kt]
                    else:
                        off = (C if is_k else 0) + j * 128
                        w = wqkv[kt][:, off:off + 128]
                    nc.tensor.matmul(
                        ps[:], w, rhs_src[kt][:, c * 512:(c + 1) * 512],
                        start=(kt == 0), stop=(kt == CT - 1))
                nc.vector.tensor_copy(dst[:, c * 512:(c + 1) * 512], ps[:])

        def v_unit(j, t):
            """v rows for token tile t, head pair j -> vaug[t][:, j*128:+128]."""
            ps = vps.tile([128, 512], F32, tag="vx", name=f"v{j}_{t}")
            for kt in range(CT):
                w = (wv0[kt] if j == 0 else
                     wqkv[kt][:, 2 * C + j * 128:2 * C + (j + 1) * 128])
                nc.tensor.matmul(
                    ps[:, 0:128], xRT[kt][:, t * 128:(t + 1) * 128], w,
                    start=(kt == 0), stop=(kt == CT - 1))
            # strided copy into the two heads' [v|1] blocks (65-stride)
            dst = vaug[t][:, 130 * j:130 * j + 130]
            dst = dst.rearrange("p (b i) -> p b i", i=65)[:, :, 0:64]
            srcv = ps[:, 0:128].rearrange("p (b i) -> p b i", i=64)
            nc.vector.tensor_copy(dst, srcv)

        fsb = [fsbp.tile([128, N], BF16, tag=f"f{i}", name=f"f{i}")
               for i in range(CT)]

        def proj_unit(oc, c):
            ps = mix.tile([128, 512], F32, tag="mx", name=f"pj{oc}{c}")
            for kt in range(NP):
                nc.tensor.matmul(
                    ps[:], pwT[kt][:, oc * 128:(oc + 1) * 128],
                    outT[kt][:, c * 512:(c + 1) * 512],
                    start=(kt == 0), stop=(kt == NP - 1))
            fh = fsb[oc][:, c * 512:(c + 1) * 512]
            nc.scalar.activation(fh, ps[:],
                                 mybir.ActivationFunctionType.Identity,
                                 bias=pbc[:, oc:oc + 1])
            nc.sync.dma_start(
                out_d.ap()[oc * 128:(oc + 1) * 128, c * 512:(c + 1) * 512], fh)

        # prefix: q0, k0
        qk_unit(0, False)
        qk_unit(0, True)

        # ---- attention pair loop (natural-layout AV) ----
        # per phase (j, c): stages compute scores/exp/P for both heads and
        # buffer the 16 P tiles; the AV chains (group-major per av region)
        # and finalize are emitted into the NEXT phase's stages.
        pending = [None]

        def av_chains(j, hi, phs, avs):
            for qc in range(4):
                for t in range(NT):
                    nc.tensor.matmul(
                        avs[hi][:, qc * 65:(qc + 1) * 65],
                        phs[hi][t][:, qc * 128:(qc + 1) * 128],
                        vaug[t][:, (2 * j + hi) * 65:(2 * j + hi + 1) * 65],
                        start=(t == 0), stop=(t == NT - 1))

        def av_fin(j, c, hi, avs, tr):
            rcp = finp.tile([128, 4], F32, tag="rcp", name=f"rcp{j}{hi}{c}")
            dn = avs[hi][:].rearrange("p (b i) -> p b i", i=65)[:, :, 64:65]
            with nc.allow_low_precision(reason="softmax reciprocal"):
                nc.vector.reciprocal(rcp[:], dn.squeeze(-1))
            for qc in range(4):
                tin = tinp.tile([128, 64], BF16, tag="tin",
                                name=f"ti{j}{hi}{c}{qc}")
                nc.vector.tensor_scalar_mul(
                    tin[:], avs[hi][:, qc * 65:qc * 65 + 64], rcp[:, qc:qc + 1])
                nc.tensor.transpose(
                    tr[hi * 64:(hi + 1) * 64, qc * 128:(qc + 1) * 128],
                    tin[:], ident[:], tile_position=(0, hi * 64))

        def av_block(j, c, phs, part=None):
            """AV chains + finalize for phase (j, c), optionally split."""
            if part in (0, None):
                avs = [avp.tile([128, 260], F32, tag="av",
                                name=f"av{j}_{hi}{c}") for hi in range(2)]
                av_block.avs = avs
                av_chains(j, 0, phs, avs)
            if part in (1, None):
                avs = av_block.avs
                tr = trp.tile([128, 512], BF16, tag="tr", name=f"tr{j}{c}")
                av_chains(j, 1, phs, avs)
                av_fin(j, c, 0, avs, tr)
                av_fin(j, c, 1, avs, tr)
                nc.vector.tensor_copy(outT[j][:, c * 512:(c + 1) * 512], tr[:])

        fetch_tbl_pair(1)
        for j in range(NP):
            if j + 2 < NP:
                fetch_tbl_pair(j + 2)
            for c in range(2):
                phs = ([], [])
                for t in range(NT):
                    if c == 0:
                        v_unit(j, t)
                    for hi in range(2):
                        pss = mix.tile([128, 512], F32, tag="mx",
                                       name=f"sc{j}{hi}{t}{c}")
                        kh = qk_sb[6 + j][hi * 64:(hi + 1) * 64,
                                          t * 128:(t + 1) * 128]
                        nc.tensor.matmul(
                            pss[:], kh,
                            qk_sb[j][hi * 64:(hi + 1) * 64,
                                     c * 512:(c + 1) * 512],
                            start=True, stop=True)
                        ee = eep.tile([128, 512], BF16, tag="ee",
                                      name=f"ee{j}{hi}{t}{c}")
                        nc.scalar.activation(ee[:], pss[:], EXP, scale=SCALE)
                        tv = tbl_tiles[2 * j + hi][
                            :, 252 * t + 1008 * c:252 * t + 1008 * c + 1008]
                        tv = tv.rearrange("p (a b) -> p a b", b=63)[:, :, :32]
                        ph = ppp.tile([128, 512], BF16, tag="ph",
                                      name=f"ph{j}{hi}{t}{c}")
                        pv = ph[:].rearrange("p (a b) -> p a b", b=32)
                        ev = ee[:].rearrange("p (a b) -> p a b", b=32)
                        nc.vector.tensor_mul(pv, ev, tv)
                        phs[hi].append(ph)
                    # previous phase's AV chains ride the mid stages
                    if pending[0] is not None:
                        if t == 1:
                            av_block(*pending[0], part=0)
                        elif t == 3:
                            av_block(*pending[0], part=1)
                            pending[0] = None
                    # next pair's q/k interleave (late stages: wqkv loaded)
                    if j + 1 < NP and c == 1:
                        if t == 4:
                            qk_unit(j + 1, False)
                        elif t == 6:
                            qk_unit(j + 1, True)
                    # last pair, c=1: interleave proj units for the c=0 half
                    if j == NP - 1 and c == 1 and 4 <= t < 4 + CT:
                        proj_unit(t - 4, 0)
                pending[0] = (j, c, phs)
        for oc in range(4, CT):
            proj_unit(oc, 0)
        av_block(*pending[0])
        pending[0] = None

        # ---- proj c=1 half (c=0 was interleaved into pair 5) ----
        for oc in range(CT):
            proj_unit(oc, 1)


_GRAPH = None


def _graph():
    global _GRAPH
    if _GRAPH is None:
        _GRAPH = _build_graph()
    return _GRAPH


def _host_prep(x, qkv_w, proj_w, proj_b, rpb_w1, rpb_b1, rpb_w2, rpb_b2):
    """Numpy layout/dtype prep + exp of the 63x63 bias table (7 MFLOP)."""
    import ml_dtypes
    a = np.arange(63, dtype=np.float32) - 31.0
    rel_y = np.broadcast_to(a[:, None], (63, 63))
    rel_x = np.broadcast_to(a[None, :], (63, 63))
    rel = np.stack([rel_x, rel_y], -1).reshape(-1, 2)           # [3969, 2]
    hdn = np.maximum(rel @ rpb_w1.T + rpb_b1, 0.0)
    gtbl = (hdn @ rpb_w2.T + rpb_b2).T.astype(np.float32)       # [12, 3969]
    gtbl = np.exp(gtbl, dtype=np.float32)                       # exp(bias)
    gpad = np.zeros((H, TW), np.float32)
    gpad[:, :3969] = gtbl
    gpad = gpad.astype(ml_dtypes.bfloat16)

    bf = ml_dtypes.bfloat16
    wqkvT = np.ascontiguousarray(qkv_w.T.astype(bf))            # [768, 2304]
    wprojT = np.ascontiguousarray(proj_w.T.astype(bf))          # [768, 768]
    pbc = np.ascontiguousarray(
        proj_b.astype(np.float32).reshape(CT, 128).T)           # [128, 6]
    shared = {"qkv_wT": wqkvT, "proj_wT": wprojT, "proj_b_col": pbc,
              "rpb_tbl": gpad}
    in_maps = []
    for i in range(B):
        m = dict(shared)
        m["xT"] = np.ascontiguousarray(x[i].T.astype(bf))
        m["xRT"] = np.ascontiguousarray(x[i][::-1].T.astype(bf))
        in_maps.append(m)
    return in_maps


def kernel(x, qkv_w, proj_w, proj_b, rpb_w1, rpb_b1, rpb_w2, rpb_b2,
           _trace=False, _tmpdir=None):
    in_maps = _host_prep(np.asarray(x), np.asarray(qkv_w), np.asarray(proj_w),
                         np.asarray(proj_b), np.asarray(rpb_w1),
                         np.asarray(rpb_b1), np.asarray(rpb_w2),
                         np.asarray(rpb_b2))
    nc = _graph()
    res = run_bass_kernel_spmd(nc, in_maps, core_ids=list(range(B)),
                               trace=_trace, tmpdir=_tmpdir)
    out = np.stack([np.ascontiguousarray(res.results[i]["out"].T)
                    for i in range(B)]).astype(np.float32)
    if _trace:
        kernel._last_results = res
    return out


# revision 38
# speedup vs baseline: 1.0019x; 1.0019x over previous
"""Multi-head attention with relative-position-bias MLP on 8 TRN2 NeuronCores.

Strategy: data-parallel over batch (B=8 -> 1 element per core, no
collectives). Host prep is layout/dtype only (bf16 casts, transposes, and
exp() of the tiny 63x63 rel-pos-bias table = ~7 MFLOP of a 66 GFLOP problem).

Design:
  - everything bf16 on SBUF (f32r matmul rate is the same; halves DMA+SBUF).
  - half-width (512-col) substages, c-half as the outer loop per head pair:
    every psum tile is a single bank -> 8 banks split as scores/qk/proj
    ring x4, AV accumulators x2, V x1, transpose staging x1, giving enough
    ring lookahead to keep PE continuously busy at max p-state.
  - natural-layout AV: out[nq, 65] matmuls with lhsT = P tile slices (half
    the AV columns of the transposed form); rhs is [v_h | ones] so column
    64 is the softmax denominator per query row. Chains are emitted
    group-major (interleaved psum accumulation groups are broken on this
    toolchain) by buffering a phase's 16 P tiles and riding the chains +
    finalize into the next phase's stages.
  - finalize: batched reciprocal (partition-aligned), per-partition
    tensor_scalar normalize to bf16, PE transpose rebuilds [c, n] for proj.
  - proj computed transposed (final^T[o,n]) so proj_b is a per-partition
    Act bias; out stored [C,N] bf16, transposed/upcast on host. The c=0
    proj half is interleaved into the last pair's stages.
  - inputs arrive as one multi-dim-AP HWDGE transfer per tensor on the SP
    queue (DMA issue holds the issuing sequencer ~0.65us, so few, large
    issues with no data-dependent waits); per-head bias tables are built
    by a single 3D-AP replicating DMA, fetched one pair ahead.
  - QKV for pair j+1 and V for pair j ride pair j's attention stages
    through the shared psum ring.
"""
import sys

import numpy as np

sys.path.insert(0, "/opt/trn_rl_repo")

import concourse.bass as bass  # noqa: E402
import concourse.mybir as mybir  # noqa: E402
import concourse.tile as tile  # noqa: E402
from concourse import bacc  # noqa: E402
from concourse.bass_utils import run_bass_kernel_spmd  # noqa: E402
from concourse.masks import make_identity  # noqa: E402

F32 = mybir.dt.float32
BF16 = mybir.dt.bfloat16
EXP = mybir.ActivationFunctionType.Exp
MULT = mybir.AluOpType.mult

B, N, C, H, D = 8, 1024, 768, 12, 64
SCALE = float(D) ** -0.5
NT = N // 128   # 8 token tiles
CT = C // 128   # 6 channel tiles
NP = H // 2     # 6 head pairs
TBLW = 3781     # replicated-table width (2016-wide views at 252*t stay in range)
TW = 4001       # DRAM table stride per head (>= 220 + TBLW, zero padded)


def _build_graph():
    nc = bacc.Bacc("TRN2", target_bir_lowering=False, debug=False,
                   enable_asserts=False, num_devices=B)
    xT_d = nc.dram_tensor("xT", [C, N], BF16, kind="ExternalInput")
    xRT_d = nc.dram_tensor("xRT", [C, N], BF16, kind="ExternalInput")
    wqkv_d = nc.dram_tensor("qkv_wT", [C, 3 * C], BF16, kind="ExternalInput")
    wproj_d = nc.dram_tensor("proj_wT", [C, C], BF16, kind="ExternalInput")
    pbc_d = nc.dram_tensor("proj_b_col", [128, CT], F32, kind="ExternalInput")
    tbl_d = nc.dram_tensor("rpb_tbl", [H, TW], BF16, kind="ExternalInput")
    out_d = nc.dram_tensor("out", [C, N], BF16, kind="ExternalOutput")

    with tile.TileContext(nc) as tc:
        _kern(tc, nc, xT_d, xRT_d, wqkv_d, wproj_d, pbc_d, tbl_d, out_d)
    nc.compile()
    return nc


def _kern(tc, nc, xT_d, xRT_d, wqkv_d, wproj_d, pbc_d, tbl_d, out_d):
    from contextlib import ExitStack

    with ExitStack() as es:
        persist = es.enter_context(tc.tile_pool(name="persist", bufs=1))
        ld = es.enter_context(tc.tile_pool(name="ld", bufs=1))
        tblp = es.enter_context(tc.tile_pool(name="tblp", bufs=6))
        eep = es.enter_context(tc.tile_pool(name="eep", bufs=8))
        ppp = es.enter_context(tc.tile_pool(name="ppp", bufs=24))
        finp = es.enter_context(tc.tile_pool(name="finp", bufs=2))
        tinp = es.enter_context(tc.tile_pool(name="tinp", bufs=12))
        fsbp = es.enter_context(tc.tile_pool(name="fsbp", bufs=1))
        # psum banks: scores/qk/proj ring 4 + av 2 + v 1 + tr 1
        mix = es.enter_context(tc.tile_pool(name="mix", bufs=4, space="PSUM"))
        avp = es.enter_context(tc.tile_pool(name="avp", bufs=2, space="PSUM"))
        vps = es.enter_context(tc.tile_pool(name="vps", bufs=1, space="PSUM"))
        trp = es.enter_context(tc.tile_pool(name="trp", bufs=1, space="PSUM"))

        # ---- persistent SBUF ----
        qk_sb = [persist.tile([128, N], BF16, tag=f"qk{i}", name=f"qk{i}")
                 for i in range(12)]   # 0..5 qT per pair, 6..11 kT per pair
        # per head 65 cols: [v(64) | ones(1)]; natural-layout AV gives
        # out[nq, 65] with col 64 = softmax denominator per query row
        vaug = [persist.tile([128, H * 65], BF16, tag=f"va{i}",
                             name=f"va{i}") for i in range(NT)]
        ident = persist.tile([128, 128], BF16, tag="ident")
        make_identity(nc, ident[:])
        warm = persist.tile([1, 1], F32, tag="warm")
        nc.vector.memset(warm[:], 0.0)
        nc.scalar.activation(warm[:], warm[:], EXP)
        outT = [persist.tile([128, N], BF16, tag=f"ot{i}", name=f"ot{i}")
                for i in range(NP)]
        pbc = persist.tile([128, CT], F32, tag="pbc")
        for t in range(NT):
            nc.gpsimd.memset(vaug[t][:], 1.0)

        # ---- input DMAs: one multi-dim-AP transfer per tensor ----
        # c-chunk-major combined tiles; all issued via SP HWDGE, no waits.
        xTt = ld.tile([128, CT * N], BF16, tag="xT")
        xRTt = ld.tile([128, CT * N], BF16, tag="xRT")
        w0q = ld.tile([128, CT * 128], BF16, tag="w0q")   # pair-0 q slices
        w0k = ld.tile([128, CT * 128], BF16, tag="w0k")
        w0v = ld.tile([128, CT * 128], BF16, tag="w0v")
        wqkvt = ld.tile([128, CT * 3 * C], BF16, tag="wqkv")
        pwTt = ld.tile([128, CT * C], BF16, tag="pwT")
        xT = [xTt[:, i * N:(i + 1) * N] for i in range(CT)]
        xRT = [xRTt[:, i * N:(i + 1) * N] for i in range(CT)]
        wqkv = [wqkvt[:, i * 3 * C:(i + 1) * 3 * C] for i in range(CT)]
        pwT = [pwTt[:, i * C:(i + 1) * C] for i in range(CT)]
        def _w0dma(sec, wt):
            nc.sync.dma_start(
                wt[:], bass.AP(wqkv_d, sec * C,
                               [[3 * C, 128], [128 * 3 * C, CT], [1, 128]]))

        nc.sync.dma_start(
            xTt[:, 0:3 * N],
            bass.AP(xT_d, 0, [[N, 128], [128 * N, 3], [1, N]]))
        _w0dma(0, w0q)
        nc.sync.dma_start(
            xTt[:, 3 * N:], bass.AP(xT_d, 3 * 128 * N,
                                    [[N, 128], [128 * N, 3], [1, N]]))
        nc.sync.dma_start(
            xRTt[:], bass.AP(xRT_d, 0, [[N, 128], [128 * N, CT], [1, N]]))
        _w0dma(1, w0k)
        _w0dma(2, w0v)
        wq0 = [w0q[:, i * 128:(i + 1) * 128] for i in range(CT)]
        wk0 = [w0k[:, i * 128:(i + 1) * 128] for i in range(CT)]
        wv0 = [w0v[:, i * 128:(i + 1) * 128] for i in range(CT)]
        # tables: one 3D-AP DMA per head, fetched one pair ahead
        tbl_tiles = {}

        def fetch_tbl_pair(j):
            for h in (2 * j, 2 * j + 1):
                t = tblp.tile([128, TBLW], BF16, tag="tbl", name=f"tbl{h}")
                nc.sync.dma_start(
                    t[:], bass.AP(tbl_d, h * TW,
                                  [[63, 4], [1, 32], [1, TBLW]]))
                tbl_tiles[h] = t

        fetch_tbl_pair(0)
        nc.sync.dma_start(
            wqkvt[:], bass.AP(wqkv_d, 0,
                              [[3 * C, 128], [128 * 3 * C, CT], [1, 3 * C]]))
        nc.sync.dma_start(
            pwTt[:], bass.AP(wproj_d, 0, [[C, 128], [128 * C, CT], [1, C]]))
        nc.sync.dma_start(pbc[:], pbc_d.ap()[:, :])

        # ---- qkv unit emitters ----
        def qk_unit(j, is_k):
            """Compute qT (or kT) for pair j into qk_sb[j (+6)]."""
            dst = qk_sb[6 + j] if is_k else qk_sb[j]
            rhs_src = xRT if is_k else xT
            for c in range(2):
                ps = mix.tile([128, 512], F32, tag="mx", name=f"qk{j}{is_k}{c}")
                for kt in range(CT):
                    if j == 0:
                        w = (wk0 if is_k else wq0)[kt]
                    else:
                        off = (C if is_k else 0) + j * 128
                        w = wqkv[kt][:, off:off + 128]
                    nc.tensor.matmul(
                        ps[:], w, rhs_src[kt][:, c * 512:(c + 1) * 512],
                        start=(kt == 0), stop=(kt == CT - 1))
                nc.vector.tensor_copy(dst[:, c * 512:(c + 1) * 512], ps[:])

        def v_unit(j, t):
            """v rows for token tile t, head pair j -> vaug[t][:, j*128:+128]."""
            ps = vps.tile([128, 512], F32, tag="vx", name=f"v{j}_{t}")
            for kt in range(CT):
                w = (wv0[kt] if j == 0 else
                     wqkv[kt][:, 2 * C + j * 128:2 * C + (j + 1) * 128])
                nc.tensor.matmul(
                    ps[:, 0:128], xRT[kt][:, t * 128:(t + 1) * 128], w,
                    start=(kt == 0), stop=(kt == CT - 1))
            # strided copy into the two heads' [v|1] blocks (65-stride)
            dst = vaug[t][:, 130 * j:130 * j + 130]
            dst = dst.rearrange("p (b i) -> p b i", i=65)[:, :, 0:64]
            srcv = ps[:, 0:128].rearrange("p (b i) -> p b i", i=64)
            nc.vector.tensor_copy(dst, srcv)

        fsb = [fsbp.tile([128, N], BF16, tag=f"f{i}", name=f"f{i}")
               for i in range(CT)]

        def proj_unit(oc, c):
            ps = mix.tile([128, 512], F32, tag="mx", name=f"pj{oc}{c}")
            for kt in range(NP):
                nc.tensor.matmul(
                    ps[:], pwT[kt][:, oc * 128:(oc + 1) * 128],
                    outT[kt][:, c * 512:(c + 1) * 512],
                    start=(kt == 0), stop=(kt == NP - 1))
            fh = fsb[oc][:, c * 512:(c + 1) * 512]
            nc.scalar.activation(fh, ps[:],
                                 mybir.ActivationFunctionType.Identity,
                                 bias=pbc[:, oc:oc + 1])
            nc.sync.dma_start(
                out_d.ap()[oc * 128:(oc + 1) * 128, c * 512:(c + 1) * 512], fh)

        # prefix: q0, k0
        qk_unit(0, False)
        qk_unit(0, True)

        # ---- attention pair loop (natural-layout AV) ----
        # per phase (j, c): stages compute scores/exp/P for both heads and
        # buffer the 16 P tiles; the AV chains (group-major per av region)
        # and finalize are emitted into the NEXT phase's stages.
        pending = [None]

        def av_chains(j, hi, phs, avs):
            for qc in range(4):
                for t in range(NT):
                    nc.tensor.matmul(
                        avs[hi][:, qc * 65:(qc + 1) * 65],
                        phs[hi][t][:, qc * 128:(qc + 1) * 128],
                        vaug[t][:, (2 * j + hi) * 65:(2 * j + hi + 1) * 65],
                        start=(t == 0), stop=(t == NT - 1))

        def av_fin(j, c, avs, tr):
            rcps = []
            for hi in range(2):
                rcp = finp.tile([128, 4], F32, tag="rcp", name=f"rcp{j}{hi}{c}")
                dn = avs[hi][:].rearrange("p (b i) -> p b i", i=65)[:, :, 64:65]
                with nc.allow_low_precision(reason="softmax reciprocal"):
                    nc.vector.reciprocal(rcp[:], dn.squeeze(-1))
                rcps.append(rcp)
            for qc in range(4):
                # both heads' normalized columns in one tile -> one [128,128]
                # transpose per qc (transpose cost is column count only)
                tin = tinp.tile([128, 128], BF16, tag="tin",
                                name=f"ti{j}{c}{qc}")
                for hi in range(2):
                    nc.vector.tensor_scalar_mul(
                        tin[:, hi * 64:(hi + 1) * 64],
                        avs[hi][:, qc * 65:qc * 65 + 64],
                        rcps[hi][:, qc:qc + 1])
                nc.tensor.transpose(
                    tr[:, qc * 128:(qc + 1) * 128], tin[:], ident[:])

        def av_block(j, c, phs, part=None):
            """AV chains + finalize for phase (j, c), optionally split."""
            if part in (0, None):
                avs = [avp.tile([128, 260], F32, tag="av",
                                name=f"av{j}_{hi}{c}") for hi in range(2)]
                av_block.avs = avs
                av_chains(j, 0, phs, avs)
            if part in (1, None):
                avs = av_block.avs
                tr = trp.tile([128, 512], BF16, tag="tr", name=f"tr{j}{c}")
                av_chains(j, 1, phs, avs)
                av_fin(j, c, avs, tr)
                nc.vector.tensor_copy(outT[j][:, c * 512:(c + 1) * 512], tr[:])

        fetch_tbl_pair(1)
        for j in range(NP):
            if j + 2 < NP:
                fetch_tbl_pair(j + 2)
            for c in range(2):
                phs = ([], [])
                for t in range(NT):
                    if c == 0:
                        v_unit(j, t)
                    for hi in range(2):
                        pss = mix.tile([128, 512], F32, tag="mx",
                                       name=f"sc{j}{hi}{t}{c}")
                        kh = qk_sb[6 + j][hi * 64:(hi + 1) * 64,
                                          t * 128:(t + 1) * 128]
                        nc.tensor.matmul(
                            pss[:], kh,
                            qk_sb[j][hi * 64:(hi + 1) * 64,
                                     c * 512:(c + 1) * 512],
                            start=True, stop=True)
                        ee = eep.tile([128, 512], BF16, tag="ee",
                                      name=f"ee{j}{hi}{t}{c}")
                        nc.scalar.activation(ee[:], pss[:], EXP, scale=SCALE)
                        tv = tbl_tiles[2 * j + hi][
                            :, 252 * t + 1008 * c:252 * t + 1008 * c + 1008]
                        tv = tv.rearrange("p (a b) -> p a b", b=63)[:, :, :32]
                        ph = ppp.tile([128, 512], BF16, tag="ph",
                                      name=f"ph{j}{hi}{t}{c}")
                        pv = ph[:].rearrange("p (a b) -> p a b", b=32)
                        ev = ee[:].rearrange("p (a b) -> p a b", b=32)
                        nc.vector.tensor_mul(pv, ev, tv)
                        phs[hi].append(ph)
                    # previous phase's AV chains ride the mid stages
                    if pending[0] is not None:
                        if t == 1:
                            av_block(*pending[0], part=0)
                        elif t == 3:
                            av_block(*pending[0], part=1)
                            pending[0] = None
                    # next pair's q/k interleave (late stages: wqkv loaded)
                    if j + 1 < NP and c == 1:
                        if t == 4:
                            qk_unit(j + 1, False)
                        elif t == 6:
                            qk_unit(j + 1, True)
                    # last pair, c=1: interleave proj units for the c=0 half
                    if j == NP - 1 and c == 1 and 4 <= t < 4 + CT:
                        proj_unit(t - 4, 0)
                pending[0] = (j, c, phs)
        for oc in range(4, CT):
            proj_unit(oc, 0)
        av_block(*pending[0])
        pending[0] = None

        # ---- proj c=1 half (c=0 was interleaved into pair 5) ----
        for oc in range(CT):
            proj_unit(oc, 1)


_GRAPH = None


def _graph():
    global _GRAPH
    if _GRAPH is None:
        _GRAPH = _build_graph()
    return _GRAPH


def _host_prep(x, qkv_w, proj_w, proj_b, rpb_w1, rpb_b1, rpb_w2, rpb_b2):
    """Numpy layout/dtype prep + exp of the 63x63 bias table (7 MFLOP)."""
    import ml_dtypes
    a = np.arange(63, dtype=np.float32) - 31.0
    rel_y = np.broadcast_to(a[:, None], (63, 63))
    rel_x = np.broadcast_to(a[None, :], (63, 63))
    rel = np.stack([rel_x, rel_y], -1).reshape(-1, 2)           # [3969, 2]
    hdn = np.maximum(rel @ rpb_w1.T + rpb_b1, 0.0)
    gtbl = (hdn @ rpb_w2.T + rpb_b2).T.astype(np.float32)       # [12, 3969]
    gtbl = np.exp(gtbl, dtype=np.float32)                       # exp(bias)
    gpad = np.zeros((H, TW), np.float32)
    gpad[:, :3969] = gtbl
    gpad = gpad.astype(ml_dtypes.bfloat16)

    bf = ml_dtypes.bfloat16
    wqkvT = np.ascontiguousarray(qkv_w.T.astype(bf))            # [768, 2304]
    wprojT = np.ascontiguousarray(proj_w.T.astype(bf))          # [768, 768]
    pbc = np.ascontiguousarray(
        proj_b.astype(np.float32).reshape(CT, 128).T)           # [128, 6]
    shared = {"qkv_wT": wqkvT, "proj_wT": wprojT, "proj_b_col": pbc,
              "rpb_tbl": gpad}
    in_maps = []
    for i in range(B):
        m = dict(shared)
        m["xT"] = np.ascontiguousarray(x[i].T.astype(bf))
        m["xRT"] = np.ascontiguousarray(x[i][::-1].T.astype(bf))
        in_maps.append(m)
    return in_maps


def kernel(x, qkv_w, proj_w, proj_b, rpb_w1, rpb_b1, rpb_w2, rpb_b2,
           _trace=False, _tmpdir=None):
    in_maps = _host_prep(np.asarray(x), np.asarray(qkv_w), np.asarray(proj_w),
                         np.asarray(proj_b), np.asarray(rpb_w1),
                         np.asarray(rpb_b1), np.asarray(rpb_w2),
                         np.asarray(rpb_b2))
    nc = _graph()
    res = run_bass_kernel_spmd(nc, in_maps, core_ids=list(range(B)),
                               trace=_trace, tmpdir=_tmpdir)
    out = np.stack([np.ascontiguousarray(res.results[i]["out"].T)
                    for i in range(B)]).astype(np.float32)
    if _trace:
        kernel._last_results = res
    return out


# revision 39
# speedup vs baseline: 1.0277x; 1.0258x over previous
"""Multi-head attention with relative-position-bias MLP on 8 TRN2 NeuronCores.

Strategy: data-parallel over batch (B=8 -> 1 element per core, no
collectives). Host prep is layout/dtype only (bf16 casts, transposes, and
exp() of the tiny 63x63 rel-pos-bias table = ~7 MFLOP of a 66 GFLOP problem).

Design:
  - everything bf16 on SBUF (f32r matmul rate is the same; halves DMA+SBUF).
  - half-width (512-col) substages, c-half as the outer loop per head pair:
    every psum tile is a single bank -> 8 banks split as scores/qk/proj
    ring x4, AV accumulators x2, V x1, transpose staging x1, giving enough
    ring lookahead to keep PE continuously busy at max p-state.
  - natural-layout AV: out[nq, 65] matmuls with lhsT = P tile slices (half
    the AV columns of the transposed form); rhs is [v_h | ones] so column
    64 is the softmax denominator per query row. Chains are emitted
    group-major (interleaved psum accumulation groups are broken on this
    toolchain) by buffering a phase's 16 P tiles and riding the chains +
    finalize into the next phase's stages.
  - finalize: batched reciprocal (partition-aligned), per-partition
    tensor_scalar normalize to bf16, PE transpose rebuilds [c, n] for proj.
  - proj computed transposed (final^T[o,n]) so proj_b is a per-partition
    Act bias; out stored [C,N] bf16, transposed/upcast on host. The c=0
    proj half is interleaved into the last pair's stages.
  - inputs arrive as one multi-dim-AP HWDGE transfer per tensor on the SP
    queue (DMA issue holds the issuing sequencer ~0.65us, so few, large
    issues with no data-dependent waits); per-head bias tables are built
    by a single 3D-AP replicating DMA, fetched one pair ahead.
  - QKV for pair j+1 and V for pair j ride pair j's attention stages
    through the shared psum ring.
"""
import sys

import numpy as np

sys.path.insert(0, "/opt/trn_rl_repo")

import concourse.bass as bass  # noqa: E402
import concourse.mybir as mybir  # noqa: E402
import concourse.tile as tile  # noqa: E402
from concourse import bacc  # noqa: E402
from concourse.bass_utils import run_bass_kernel_spmd  # noqa: E402
from concourse.masks import make_identity  # noqa: E402

F32 = mybir.dt.float32
BF16 = mybir.dt.bfloat16
EXP = mybir.ActivationFunctionType.Exp
MULT = mybir.AluOpType.mult

B, N, C, H, D = 8, 1024, 768, 12, 64
SCALE = float(D) ** -0.5
NT = N // 128   # 8 token tiles
CT = C // 128   # 6 channel tiles
NP = H // 2     # 6 head pairs
TBLW = 3781     # replicated-table width (2016-wide views at 252*t stay in range)
TW = 4001       # DRAM table stride per head (>= 220 + TBLW, zero padded)


def _build_graph():
    nc = bacc.Bacc("TRN2", target_bir_lowering=False, debug=False,
                   enable_asserts=False, num_devices=B)
    xT_d = nc.dram_tensor("xT", [C, N], BF16, kind="ExternalInput")
    xRT_d = nc.dram_tensor("xRT", [C, N], BF16, kind="ExternalInput")
    wqkv_d = nc.dram_tensor("qkv_wT", [C, 3 * C], BF16, kind="ExternalInput")
    wproj_d = nc.dram_tensor("proj_wT", [C, C], BF16, kind="ExternalInput")
    pbc_d = nc.dram_tensor("proj_b_col", [128, CT], F32, kind="ExternalInput")
    tbl_d = nc.dram_tensor("rpb_tbl", [H, TW], BF16, kind="ExternalInput")
    out_d = nc.dram_tensor("out", [C, N], BF16, kind="ExternalOutput")

    with tile.TileContext(nc) as tc:
        _kern(tc, nc, xT_d, xRT_d, wqkv_d, wproj_d, pbc_d, tbl_d, out_d)
    nc.compile()
    return nc


def _kern(tc, nc, xT_d, xRT_d, wqkv_d, wproj_d, pbc_d, tbl_d, out_d):
    from contextlib import ExitStack

    with ExitStack() as es:
        persist = es.enter_context(tc.tile_pool(name="persist", bufs=1))
        ld = es.enter_context(tc.tile_pool(name="ld", bufs=1))
        tblp = es.enter_context(tc.tile_pool(name="tblp", bufs=6))
        eep = es.enter_context(tc.tile_pool(name="eep", bufs=8))
        ppp = es.enter_context(tc.tile_pool(name="ppp", bufs=24))
        finp = es.enter_context(tc.tile_pool(name="finp", bufs=2))
        tinp = es.enter_context(tc.tile_pool(name="tinp", bufs=12))
        fsbp = es.enter_context(tc.tile_pool(name="fsbp", bufs=1))
        # psum banks: scores/qk/proj ring 4 + av 2 + v 1 + tr 1
        mix = es.enter_context(tc.tile_pool(name="mix", bufs=4, space="PSUM"))
        avp = es.enter_context(tc.tile_pool(name="avp", bufs=2, space="PSUM"))
        vps = es.enter_context(tc.tile_pool(name="vps", bufs=1, space="PSUM"))
        trp = es.enter_context(tc.tile_pool(name="trp", bufs=1, space="PSUM"))

        # ---- persistent SBUF ----
        qk_sb = [persist.tile([128, N], BF16, tag=f"qk{i}", name=f"qk{i}")
                 for i in range(12)]   # 0..5 qT per pair, 6..11 kT per pair
        # per head 65 cols: [v(64) | ones(1)]; natural-layout AV gives
        # out[nq, 65] with col 64 = softmax denominator per query row
        vaug = [persist.tile([128, H * 65], BF16, tag=f"va{i}",
                             name=f"va{i}") for i in range(NT)]
        ident = persist.tile([128, 128], BF16, tag="ident")
        make_identity(nc, ident[:])
        warm = persist.tile([1, 1], F32, tag="warm")
        nc.vector.memset(warm[:], 0.0)
        nc.scalar.activation(warm[:], warm[:], EXP)
        outT = [persist.tile([128, N], BF16, tag=f"ot{i}", name=f"ot{i}")
                for i in range(NP)]
        pbc = persist.tile([128, CT], F32, tag="pbc")
        for t in range(NT):
            nc.gpsimd.memset(vaug[t][:], 1.0)

        # ---- input DMAs: one multi-dim-AP transfer per tensor ----
        # c-chunk-major combined tiles; all issued via SP HWDGE, no waits.
        xTt = ld.tile([128, CT * N], BF16, tag="xT")
        xRTt = ld.tile([128, CT * N], BF16, tag="xRT")
        w0q = ld.tile([128, CT * 128], BF16, tag="w0q")   # pair-0 q slices
        w0k = ld.tile([128, CT * 128], BF16, tag="w0k")
        w0v = ld.tile([128, CT * 128], BF16, tag="w0v")
        wqkvt = ld.tile([128, CT * 3 * C], BF16, tag="wqkv")
        pwTt = ld.tile([128, CT * C], BF16, tag="pwT")
        xT = [xTt[:, i * N:(i + 1) * N] for i in range(CT)]
        xRT = [xRTt[:, i * N:(i + 1) * N] for i in range(CT)]
        wqkv = [wqkvt[:, i * 3 * C:(i + 1) * 3 * C] for i in range(CT)]
        pwT = [pwTt[:, i * C:(i + 1) * C] for i in range(CT)]
        def _w0dma(sec, wt):
            nc.sync.dma_start(
                wt[:], bass.AP(wqkv_d, sec * C,
                               [[3 * C, 128], [128 * 3 * C, CT], [1, 128]]))

        nc.sync.dma_start(
            xTt[:, 0:3 * N],
            bass.AP(xT_d, 0, [[N, 128], [128 * N, 3], [1, N]]))
        _w0dma(0, w0q)
        nc.sync.dma_start(
            xTt[:, 3 * N:], bass.AP(xT_d, 3 * 128 * N,
                                    [[N, 128], [128 * N, 3], [1, N]]))
        nc.sync.dma_start(
            xRTt[:], bass.AP(xRT_d, 0, [[N, 128], [128 * N, CT], [1, N]]))
        _w0dma(1, w0k)
        _w0dma(2, w0v)
        wq0 = [w0q[:, i * 128:(i + 1) * 128] for i in range(CT)]
        wk0 = [w0k[:, i * 128:(i + 1) * 128] for i in range(CT)]
        wv0 = [w0v[:, i * 128:(i + 1) * 128] for i in range(CT)]
        # tables: one 3D-AP DMA per head, fetched one pair ahead
        tbl_tiles = {}

        def fetch_tbl_pair(j):
            for h in (2 * j, 2 * j + 1):
                t = tblp.tile([128, TBLW], BF16, tag="tbl", name=f"tbl{h}")
                nc.sync.dma_start(
                    t[:], bass.AP(tbl_d, h * TW,
                                  [[63, 4], [1, 32], [1, TBLW]]))
                tbl_tiles[h] = t

        fetch_tbl_pair(0)
        nc.sync.dma_start(
            wqkvt[:], bass.AP(wqkv_d, 0,
                              [[3 * C, 128], [128 * 3 * C, CT], [1, 3 * C]]))
        nc.sync.dma_start(
            pwTt[:], bass.AP(wproj_d, 0, [[C, 128], [128 * C, CT], [1, C]]))
        nc.sync.dma_start(pbc[:], pbc_d.ap()[:, :])

        # ---- qkv unit emitters ----
        def qk_half(j, is_k, c):
            dst = qk_sb[6 + j] if is_k else qk_sb[j]
            rhs_src = xRT if is_k else xT
            ps = mix.tile([128, 512], F32, tag="mx", name=f"qk{j}{is_k}{c}")
            for kt in range(CT):
                if j == 0:
                    w = (wk0 if is_k else wq0)[kt]
                else:
                    off = (C if is_k else 0) + j * 128
                    w = wqkv[kt][:, off:off + 128]
                nc.tensor.matmul(
                    ps[:], w, rhs_src[kt][:, c * 512:(c + 1) * 512],
                    start=(kt == 0), stop=(kt == CT - 1))
            nc.vector.tensor_copy(dst[:, c * 512:(c + 1) * 512], ps[:])

        def qk_unit(j, is_k):
            """Compute qT (or kT) for pair j into qk_sb[j (+6)]."""
            for c in range(2):
                qk_half(j, is_k, c)

        def v_unit(j, t):
            """v rows for token tile t, head pair j -> vaug[t][:, j*128:+128]."""
            ps = vps.tile([128, 512], F32, tag="vx", name=f"v{j}_{t}")
            for kt in range(CT):
                w = (wv0[kt] if j == 0 else
                     wqkv[kt][:, 2 * C + j * 128:2 * C + (j + 1) * 128])
                nc.tensor.matmul(
                    ps[:, 0:128], xRT[kt][:, t * 128:(t + 1) * 128], w,
                    start=(kt == 0), stop=(kt == CT - 1))
            # strided copy into the two heads' [v|1] blocks (65-stride)
            dst = vaug[t][:, 130 * j:130 * j + 130]
            dst = dst.rearrange("p (b i) -> p b i", i=65)[:, :, 0:64]
            srcv = ps[:, 0:128].rearrange("p (b i) -> p b i", i=64)
            nc.vector.tensor_copy(dst, srcv)

        fsb = [fsbp.tile([128, N], BF16, tag=f"f{i}", name=f"f{i}")
               for i in range(CT)]

        def proj_unit(oc, c):
            ps = mix.tile([128, 512], F32, tag="mx", name=f"pj{oc}{c}")
            for kt in range(NP):
                nc.tensor.matmul(
                    ps[:], pwT[kt][:, oc * 128:(oc + 1) * 128],
                    outT[kt][:, c * 512:(c + 1) * 512],
                    start=(kt == 0), stop=(kt == NP - 1))
            fh = fsb[oc][:, c * 512:(c + 1) * 512]
            nc.scalar.activation(fh, ps[:],
                                 mybir.ActivationFunctionType.Identity,
                                 bias=pbc[:, oc:oc + 1])
            nc.sync.dma_start(
                out_d.ap()[oc * 128:(oc + 1) * 128, c * 512:(c + 1) * 512], fh)

        # prefix: q0, k0
        qk_unit(0, False)
        qk_unit(0, True)

        # ---- attention pair loop (natural-layout AV) ----
        # per phase (j, c): stages compute scores/exp/P for both heads and
        # buffer the 16 P tiles; the AV chains (group-major per av region)
        # and finalize are emitted into the NEXT phase's stages.
        pending = [None]

        def av_chains(j, hi, phs, avs):
            for qc in range(4):
                for t in range(NT):
                    nc.tensor.matmul(
                        avs[hi][:, qc * 65:(qc + 1) * 65],
                        phs[hi][t][:, qc * 128:(qc + 1) * 128],
                        vaug[t][:, (2 * j + hi) * 65:(2 * j + hi + 1) * 65],
                        start=(t == 0), stop=(t == NT - 1))

        def av_fin(j, c, avs, tr):
            rcps = []
            for hi in range(2):
                rcp = finp.tile([128, 4], F32, tag="rcp", name=f"rcp{j}{hi}{c}")
                dn = avs[hi][:].rearrange("p (b i) -> p b i", i=65)[:, :, 64:65]
                with nc.allow_low_precision(reason="softmax reciprocal"):
                    nc.vector.reciprocal(rcp[:], dn.squeeze(-1))
                rcps.append(rcp)
            for qc in range(4):
                # both heads' normalized columns in one tile -> one [128,128]
                # transpose per qc (transpose cost is column count only)
                tin = tinp.tile([128, 128], BF16, tag="tin",
                                name=f"ti{j}{c}{qc}")
                for hi in range(2):
                    nc.vector.tensor_scalar_mul(
                        tin[:, hi * 64:(hi + 1) * 64],
                        avs[hi][:, qc * 65:qc * 65 + 64],
                        rcps[hi][:, qc:qc + 1])
                nc.tensor.transpose(
                    tr[:, qc * 128:(qc + 1) * 128], tin[:], ident[:])

        def av_block(j, c, phs, part=None):
            """AV chains + finalize for phase (j, c), optionally split."""
            if part in (0, None):
                avs = [avp.tile([128, 260], F32, tag="av",
                                name=f"av{j}_{hi}{c}") for hi in range(2)]
                av_block.avs = avs
                av_chains(j, 0, phs, avs)
            if part in (1, None):
                avs = av_block.avs
                tr = trp.tile([128, 512], BF16, tag="tr", name=f"tr{j}{c}")
                av_chains(j, 1, phs, avs)
                av_fin(j, c, avs, tr)
                nc.vector.tensor_copy(outT[j][:, c * 512:(c + 1) * 512], tr[:])

        fetch_tbl_pair(1)
        for j in range(NP):
            if j + 2 < NP:
                fetch_tbl_pair(j + 2)
            for c in range(2):
                phs = ([], [])
                for t in range(NT):
                    if c == 0:
                        v_unit(j, t)
                    for hi in range(2):
                        pss = mix.tile([128, 512], F32, tag="mx",
                                       name=f"sc{j}{hi}{t}{c}")
                        kh = qk_sb[6 + j][hi * 64:(hi + 1) * 64,
                                          t * 128:(t + 1) * 128]
                        nc.tensor.matmul(
                            pss[:], kh,
                            qk_sb[j][hi * 64:(hi + 1) * 64,
                                     c * 512:(c + 1) * 512],
                            start=True, stop=True)
                        ee = eep.tile([128, 512], BF16, tag="ee",
                                      name=f"ee{j}{hi}{t}{c}")
                        nc.scalar.activation(ee[:], pss[:], EXP, scale=SCALE)
                        tv = tbl_tiles[2 * j + hi][
                            :, 252 * t + 1008 * c:252 * t + 1008 * c + 1008]
                        tv = tv.rearrange("p (a b) -> p a b", b=63)[:, :, :32]
                        ph = ppp.tile([128, 512], BF16, tag="ph",
                                      name=f"ph{j}{hi}{t}{c}")
                        pv = ph[:].rearrange("p (a b) -> p a b", b=32)
                        ev = ee[:].rearrange("p (a b) -> p a b", b=32)
                        nc.vector.tensor_mul(pv, ev, tv)
                        phs[hi].append(ph)
                    # previous phase's AV chains ride the mid stages
                    if pending[0] is not None:
                        if t == 1:
                            av_block(*pending[0], part=0)
                        elif t == 3:
                            av_block(*pending[0], part=1)
                            pending[0] = None
                    # next pair's q/k interleave (late stages: wqkv loaded)
                    if j + 1 < NP and c == 1 and 4 <= t < 8:
                        qk_half(j + 1, t >= 6, t % 2)
                    # last pair, c=1: interleave proj units for the c=0 half
                    if j == NP - 1 and c == 1 and 4 <= t < 4 + CT:
                        proj_unit(t - 4, 0)
                pending[0] = (j, c, phs)
        for oc in range(4, CT):
            proj_unit(oc, 0)
        av_block(*pending[0])
        pending[0] = None

        # ---- proj c=1 half (c=0 was interleaved into pair 5) ----
        for oc in range(CT):
            proj_unit(oc, 1)


_GRAPH = None


def _graph():
    global _GRAPH
    if _GRAPH is None:
        _GRAPH = _build_graph()
    return _GRAPH


def _host_prep(x, qkv_w, proj_w, proj_b, rpb_w1, rpb_b1, rpb_w2, rpb_b2):
    """Numpy layout/dtype prep + exp of the 63x63 bias table (7 MFLOP)."""
    import ml_dtypes
    a = np.arange(63, dtype=np.float32) - 31.0
    rel_y = np.broadcast_to(a[:, None], (63, 63))
    rel_x = np.broadcast_to(a[None, :], (63, 63))
    rel = np.stack([rel_x, rel_y], -1).reshape(-1, 2)           # [3969, 2]
    hdn = np.maximum(rel @ rpb_w1.T + rpb_b1, 0.0)
    gtbl = (hdn @ rpb_w2.T + rpb_b2).T.astype(np.float32)       # [12, 3969]
    gtbl = np.exp(gtbl, dtype=np.float32)                       # exp(bias)
    gpad = np.zeros((H, TW), np.float32)
    gpad[:, :3969] = gtbl
    gpad = gpad.astype(ml_dtypes.bfloat16)

    bf = ml_dtypes.bfloat16
    wqkvT = np.ascontiguousarray(qkv_w.T.astype(bf))            # [768, 2304]
    wprojT = np.ascontiguousarray(proj_w.T.astype(bf))          # [768, 768]
    pbc = np.ascontiguousarray(
        proj_b.astype(np.float32).reshape(CT, 128).T)           # [128, 6]
    shared = {"qkv_wT": wqkvT, "proj_wT": wprojT, "proj_b_col": pbc,
              "rpb_tbl": gpad}
    in_maps = []
    for i in range(B):
        m = dict(shared)
        m["xT"] = np.ascontiguousarray(x[i].T.astype(bf))
        m["xRT"] = np.ascontiguousarray(x[i][::-1].T.astype(bf))
        in_maps.append(m)
    return in_maps


def kernel(x, qkv_w, proj_w, proj_b, rpb_w1, rpb_b1, rpb_w2, rpb_b2,
           _trace=False, _tmpdir=None):
    in_maps = _host_prep(np.asarray(x), np.asarray(qkv_w), np.asarray(proj_w),
                         np.asarray(proj_b), np.asarray(rpb_w1),
                         np.asarray(rpb_b1), np.asarray(rpb_w2),
                         np.asarray(rpb_b2))
    nc = _graph()
    res = run_bass_kernel_spmd(nc, in_maps, core_ids=list(range(B)),
                               trace=_trace, tmpdir=_tmpdir)
    out = np.stack([np.ascontiguousarray(res.results[i]["out"].T)
                    for i in range(B)]).astype(np.float32)
    if _trace:
        kernel._last_results = res
    return out


# revision 40
# speedup vs baseline: 1.0305x; 1.0027x over previous
"""Multi-head attention with relative-position-bias MLP on 8 TRN2 NeuronCores.

Strategy: data-parallel over batch (B=8 -> 1 element per core, no
collectives). Host prep is layout/dtype only (bf16 casts, transposes, and
exp() of the tiny 63x63 rel-pos-bias table = ~7 MFLOP of a 66 GFLOP problem).

Design:
  - everything bf16 on SBUF (f32r matmul rate is the same; halves DMA+SBUF).
  - half-width (512-col) substages, c-half as the outer loop per head pair:
    every psum tile is a single bank -> 8 banks split as scores/qk/proj
    ring x4, AV accumulators x2, V x1, transpose staging x1, giving enough
    ring lookahead to keep PE continuously busy at max p-state.
  - natural-layout AV: out[nq, 65] matmuls with lhsT = P tile slices (half
    the AV columns of the transposed form); rhs is [v_h | ones] so column
    64 is the softmax denominator per query row. Chains are emitted
    group-major (interleaved psum accumulation groups are broken on this
    toolchain) by buffering a phase's 16 P tiles and riding the chains +
    finalize into the next phase's stages.
  - finalize: batched reciprocal (partition-aligned), per-partition
    tensor_scalar normalize to bf16, PE transpose rebuilds [c, n] for proj.
  - proj computed transposed (final^T[o,n]) so proj_b is a per-partition
    Act bias; out stored [C,N] bf16, transposed/upcast on host. The c=0
    proj half is interleaved into the last pair's stages.
  - inputs arrive as one multi-dim-AP HWDGE transfer per tensor on the SP
    queue (DMA issue holds the issuing sequencer ~0.65us, so few, large
    issues with no data-dependent waits); per-head bias tables are built
    by a single 3D-AP replicating DMA, fetched one pair ahead.
  - QKV for pair j+1 and V for pair j ride pair j's attention stages
    through the shared psum ring.
"""
import sys

import numpy as np

sys.path.insert(0, "/opt/trn_rl_repo")

import concourse.bass as bass  # noqa: E402
import concourse.mybir as mybir  # noqa: E402
import concourse.tile as tile  # noqa: E402
from concourse import bacc  # noqa: E402
from concourse.bass_utils import run_bass_kernel_spmd  # noqa: E402
from concourse.masks import make_identity  # noqa: E402

F32 = mybir.dt.float32
BF16 = mybir.dt.bfloat16
EXP = mybir.ActivationFunctionType.Exp
MULT = mybir.AluOpType.mult

B, N, C, H, D = 8, 1024, 768, 12, 64
SCALE = float(D) ** -0.5
NT = N // 128   # 8 token tiles
CT = C // 128   # 6 channel tiles
NP = H // 2     # 6 head pairs
TBLW = 3781     # replicated-table width (2016-wide views at 252*t stay in range)
TW = 4001       # DRAM table stride per head (>= 220 + TBLW, zero padded)


def _build_graph():
    nc = bacc.Bacc("TRN2", target_bir_lowering=False, debug=False,
                   enable_asserts=False, num_devices=B)
    xT_d = nc.dram_tensor("xT", [C, N], BF16, kind="ExternalInput")
    xRT_d = nc.dram_tensor("xRT", [C, N], BF16, kind="ExternalInput")
    wqkv_d = nc.dram_tensor("qkv_wT", [C, 3 * C], BF16, kind="ExternalInput")
    wproj_d = nc.dram_tensor("proj_wT", [C, C], BF16, kind="ExternalInput")
    pbc_d = nc.dram_tensor("proj_b_col", [128, CT], F32, kind="ExternalInput")
    tbl_d = nc.dram_tensor("rpb_tbl", [H, TW], BF16, kind="ExternalInput")
    out_d = nc.dram_tensor("out", [C, N], BF16, kind="ExternalOutput")

    with tile.TileContext(nc) as tc:
        _kern(tc, nc, xT_d, xRT_d, wqkv_d, wproj_d, pbc_d, tbl_d, out_d)
    nc.compile()
    return nc


def _kern(tc, nc, xT_d, xRT_d, wqkv_d, wproj_d, pbc_d, tbl_d, out_d):
    from contextlib import ExitStack

    with ExitStack() as es:
        persist = es.enter_context(tc.tile_pool(name="persist", bufs=1))
        ld = es.enter_context(tc.tile_pool(name="ld", bufs=1))
        tblp = es.enter_context(tc.tile_pool(name="tblp", bufs=6))
        eep = es.enter_context(tc.tile_pool(name="eep", bufs=8))
        ppp = es.enter_context(tc.tile_pool(name="ppp", bufs=24))
        finp = es.enter_context(tc.tile_pool(name="finp", bufs=2))
        tinp = es.enter_context(tc.tile_pool(name="tinp", bufs=12))
        fsbp = es.enter_context(tc.tile_pool(name="fsbp", bufs=1))
        # psum banks: scores/qk/proj ring 4 + av 2 + v 1 + tr 1
        mix = es.enter_context(tc.tile_pool(name="mix", bufs=4, space="PSUM"))
        avp = es.enter_context(tc.tile_pool(name="avp", bufs=2, space="PSUM"))
        vps = es.enter_context(tc.tile_pool(name="vps", bufs=1, space="PSUM"))
        trp = es.enter_context(tc.tile_pool(name="trp", bufs=1, space="PSUM"))

        # ---- persistent SBUF ----
        qk_sb = [persist.tile([128, N], BF16, tag=f"qk{i}", name=f"qk{i}")
                 for i in range(12)]   # 0..5 qT per pair, 6..11 kT per pair
        # per head 65 cols: [v(64) | ones(1)]; natural-layout AV gives
        # out[nq, 65] with col 64 = softmax denominator per query row
        vaug = [persist.tile([128, H * 65], BF16, tag=f"va{i}",
                             name=f"va{i}") for i in range(NT)]
        ident = persist.tile([128, 128], BF16, tag="ident")
        make_identity(nc, ident[:])
        warm = persist.tile([1, 1], F32, tag="warm")
        nc.vector.memset(warm[:], 0.0)
        nc.scalar.activation(warm[:], warm[:], EXP)
        outT = [persist.tile([128, N], BF16, tag=f"ot{i}", name=f"ot{i}")
                for i in range(NP)]
        pbc = persist.tile([128, CT], F32, tag="pbc")
        for t in range(NT):
            nc.gpsimd.memset(vaug[t][:], 1.0)

        # ---- input DMAs: one multi-dim-AP transfer per tensor ----
        # c-chunk-major combined tiles; all issued via SP HWDGE, no waits.
        xTt = ld.tile([128, CT * N], BF16, tag="xT")
        xRTt = ld.tile([128, CT * N], BF16, tag="xRT")
        w0q = ld.tile([128, CT * 128], BF16, tag="w0q")   # pair-0 q slices
        w0k = ld.tile([128, CT * 128], BF16, tag="w0k")
        w0v = ld.tile([128, CT * 128], BF16, tag="w0v")
        wqkvt = ld.tile([128, CT * 3 * C], BF16, tag="wqkv")
        pwTt = ld.tile([128, CT * C], BF16, tag="pwT")
        xT = [xTt[:, i * N:(i + 1) * N] for i in range(CT)]
        xRT = [xRTt[:, i * N:(i + 1) * N] for i in range(CT)]
        wqkv = [wqkvt[:, i * 3 * C:(i + 1) * 3 * C] for i in range(CT)]
        pwT = [pwTt[:, i * C:(i + 1) * C] for i in range(CT)]
        def _w0dma(sec, wt):
            nc.sync.dma_start(
                wt[:], bass.AP(wqkv_d, sec * C,
                               [[3 * C, 128], [128 * 3 * C, CT], [1, 128]]))

        nc.sync.dma_start(
            xTt[:, 0:3 * N],
            bass.AP(xT_d, 0, [[N, 128], [128 * N, 3], [1, N]]))
        _w0dma(0, w0q)
        nc.sync.dma_start(
            xTt[:, 3 * N:], bass.AP(xT_d, 3 * 128 * N,
                                    [[N, 128], [128 * N, 3], [1, N]]))
        nc.sync.dma_start(
            xRTt[:], bass.AP(xRT_d, 0, [[N, 128], [128 * N, CT], [1, N]]))
        _w0dma(1, w0k)
        _w0dma(2, w0v)
        wq0 = [w0q[:, i * 128:(i + 1) * 128] for i in range(CT)]
        wk0 = [w0k[:, i * 128:(i + 1) * 128] for i in range(CT)]
        wv0 = [w0v[:, i * 128:(i + 1) * 128] for i in range(CT)]
        # tables: one 3D-AP DMA per head, fetched one pair ahead
        tbl_tiles = {}

        def fetch_tbl_pair(j):
            for h in (2 * j, 2 * j + 1):
                t = tblp.tile([128, TBLW], BF16, tag="tbl", name=f"tbl{h}")
                nc.sync.dma_start(
                    t[:], bass.AP(tbl_d, h * TW,
                                  [[63, 4], [1, 32], [1, TBLW]]))
                tbl_tiles[h] = t

        fetch_tbl_pair(0)
        nc.sync.dma_start(
            wqkvt[:], bass.AP(wqkv_d, 0,
                              [[3 * C, 128], [128 * 3 * C, CT], [1, 3 * C]]))
        nc.sync.dma_start(
            pwTt[:], bass.AP(wproj_d, 0, [[C, 128], [128 * C, CT], [1, C]]))
        nc.sync.dma_start(pbc[:], pbc_d.ap()[:, :])

        # ---- qkv unit emitters ----
        def qk_half(j, is_k, c):
            dst = qk_sb[6 + j] if is_k else qk_sb[j]
            rhs_src = xRT if is_k else xT
            ps = mix.tile([128, 512], F32, tag="mx", name=f"qk{j}{is_k}{c}")
            for kt in range(CT):
                if j == 0:
                    w = (wk0 if is_k else wq0)[kt]
                else:
                    off = (C if is_k else 0) + j * 128
                    w = wqkv[kt][:, off:off + 128]
                nc.tensor.matmul(
                    ps[:], w, rhs_src[kt][:, c * 512:(c + 1) * 512],
                    start=(kt == 0), stop=(kt == CT - 1))
            nc.vector.tensor_copy(dst[:, c * 512:(c + 1) * 512], ps[:])

        def qk_unit(j, is_k):
            """Compute qT (or kT) for pair j into qk_sb[j (+6)]."""
            for c in range(2):
                qk_half(j, is_k, c)

        def v_unit(j, t):
            """v rows for token tile t, head pair j -> vaug[t][:, j*128:+128]."""
            ps = vps.tile([128, 512], F32, tag="vx", name=f"v{j}_{t}")
            for kt in range(CT):
                w = (wv0[kt] if j == 0 else
                     wqkv[kt][:, 2 * C + j * 128:2 * C + (j + 1) * 128])
                nc.tensor.matmul(
                    ps[:, 0:128], xRT[kt][:, t * 128:(t + 1) * 128], w,
                    start=(kt == 0), stop=(kt == CT - 1))
            # strided copy into the two heads' [v|1] blocks (65-stride)
            dst = vaug[t][:, 130 * j:130 * j + 130]
            dst = dst.rearrange("p (b i) -> p b i", i=65)[:, :, 0:64]
            srcv = ps[:, 0:128].rearrange("p (b i) -> p b i", i=64)
            nc.vector.tensor_copy(dst, srcv)

        fsb = [fsbp.tile([128, N], BF16, tag=f"f{i}", name=f"f{i}")
               for i in range(CT)]

        def proj_unit(oc, c):
            ps = mix.tile([128, 512], F32, tag="mx", name=f"pj{oc}{c}")
            for kt in range(NP):
                nc.tensor.matmul(
                    ps[:], pwT[kt][:, oc * 128:(oc + 1) * 128],
                    outT[kt][:, c * 512:(c + 1) * 512],
                    start=(kt == 0), stop=(kt == NP - 1))
            fh = fsb[oc][:, c * 512:(c + 1) * 512]
            nc.scalar.activation(fh, ps[:],
                                 mybir.ActivationFunctionType.Identity,
                                 bias=pbc[:, oc:oc + 1])
            nc.sync.dma_start(
                out_d.ap()[oc * 128:(oc + 1) * 128, c * 512:(c + 1) * 512], fh)

        # prefix: q0, k0
        qk_unit(0, False)
        qk_unit(0, True)

        # ---- attention pair loop (natural-layout AV) ----
        # per phase (j, c): stages compute scores/exp/P for both heads and
        # buffer the 16 P tiles; the AV chains (group-major per av region)
        # and finalize are emitted into the NEXT phase's stages.
        pending = [None]

        def av_chains(j, hi, phs, avs):
            for qc in range(4):
                for t in range(NT):
                    nc.tensor.matmul(
                        avs[hi][:, qc * 65:(qc + 1) * 65],
                        phs[hi][t][:, qc * 128:(qc + 1) * 128],
                        vaug[t][:, (2 * j + hi) * 65:(2 * j + hi + 1) * 65],
                        start=(t == 0), stop=(t == NT - 1))

        def av_fin(j, c, avs, tr):
            rcps = []
            for hi in range(2):
                rcp = finp.tile([128, 4], F32, tag="rcp", name=f"rcp{j}{hi}{c}")
                dn = avs[hi][:].rearrange("p (b i) -> p b i", i=65)[:, :, 64:65]
                with nc.allow_low_precision(reason="softmax reciprocal"):
                    nc.vector.reciprocal(rcp[:], dn.squeeze(-1))
                rcps.append(rcp)
            for qc in range(4):
                # both heads' normalized columns in one tile -> one [128,128]
                # transpose per qc (transpose cost is column count only)
                tin = tinp.tile([128, 128], BF16, tag="tin",
                                name=f"ti{j}{c}{qc}")
                for hi in range(2):
                    nc.vector.tensor_scalar_mul(
                        tin[:, hi * 64:(hi + 1) * 64],
                        avs[hi][:, qc * 65:qc * 65 + 64],
                        rcps[hi][:, qc:qc + 1])
                nc.tensor.transpose(
                    tr[:, qc * 128:(qc + 1) * 128], tin[:], ident[:])

        def av_block(j, c, phs, part=None):
            """AV chains + finalize for phase (j, c), optionally split."""
            if part in (0, None):
                avs = [avp.tile([128, 260], F32, tag="av",
                                name=f"av{j}_{hi}{c}") for hi in range(2)]
                av_block.avs = avs
                av_chains(j, 0, phs, avs)
            if part in (1, None):
                avs = av_block.avs
                tr = trp.tile([128, 512], BF16, tag="tr", name=f"tr{j}{c}")
                av_chains(j, 1, phs, avs)
                av_fin(j, c, avs, tr)
                nc.vector.tensor_copy(outT[j][:, c * 512:(c + 1) * 512], tr[:])

        fetch_tbl_pair(1)
        for j in range(NP):
            if j + 2 < NP:
                fetch_tbl_pair(j + 2)
            for c in range(2):
                phs = ([], [])
                for t in range(NT):
                    if c == 0:
                        v_unit(j, t)
                    for hi in range(2):
                        pss = mix.tile([128, 512], F32, tag="mx",
                                       name=f"sc{j}{hi}{t}{c}")
                        kh = qk_sb[6 + j][hi * 64:(hi + 1) * 64,
                                          t * 128:(t + 1) * 128]
                        nc.tensor.matmul(
                            pss[:], kh,
                            qk_sb[j][hi * 64:(hi + 1) * 64,
                                     c * 512:(c + 1) * 512],
                            start=True, stop=True)
                        ee = eep.tile([128, 512], BF16, tag="ee",
                                      name=f"ee{j}{hi}{t}{c}")
                        nc.scalar.activation(ee[:], pss[:], EXP, scale=SCALE)
                        tv = tbl_tiles[2 * j + hi][
                            :, 252 * t + 1008 * c:252 * t + 1008 * c + 1008]
                        tv = tv.rearrange("p (a b) -> p a b", b=63)[:, :, :32]
                        ph = ppp.tile([128, 512], BF16, tag="ph",
                                      name=f"ph{j}{hi}{t}{c}")
                        pv = ph[:].rearrange("p (a b) -> p a b", b=32)
                        ev = ee[:].rearrange("p (a b) -> p a b", b=32)
                        nc.vector.tensor_mul(pv, ev, tv)
                        phs[hi].append(ph)
                    # previous phase's AV chains ride the mid stages
                    if pending[0] is not None:
                        if t == 1:
                            av_block(*pending[0], part=0)
                        elif t == 3:
                            av_block(*pending[0], part=1)
                            pending[0] = None
                    # next pair's q/k interleave (late stages: wqkv loaded)
                    if j + 1 < NP and c == 1 and 4 <= t < 8:
                        qk_half(j + 1, t >= 6, t % 2)
                    # last pair, c=1: interleave proj units for the c=0 half
                    if j == NP - 1 and c == 1 and 3 <= t < 3 + CT:
                        proj_unit(t - 3, 0)
                pending[0] = (j, c, phs)
        for oc in range(5, CT):
            proj_unit(oc, 0)
        av_block(*pending[0])
        pending[0] = None

        # ---- proj c=1 half (c=0 was interleaved into pair 5) ----
        for oc in range(CT):
            proj_unit(oc, 1)


_GRAPH = None


def _graph():
    global _GRAPH
    if _GRAPH is None:
        _GRAPH = _build_graph()
    return _GRAPH


def _host_prep(x, qkv_w, proj_w, proj_b, rpb_w1, rpb_b1, rpb_w2, rpb_b2):
    """Numpy layout/dtype prep + exp of the 63x63 bias table (7 MFLOP)."""
    import ml_dtypes
    a = np.arange(63, dtype=np.float32) - 31.0
    rel_y = np.broadcast_to(a[:, None], (63, 63))
    rel_x = np.broadcast_to(a[None, :], (63, 63))
    rel = np.stack([rel_x, rel_y], -1).reshape(-1, 2)           # [3969, 2]
    hdn = np.maximum(rel @ rpb_w1.T + rpb_b1, 0.0)
    gtbl = (hdn @ rpb_w2.T + rpb_b2).T.astype(np.float32)       # [12, 3969]
    gtbl = np.exp(gtbl, dtype=np.float32)                       # exp(bias)
    gpad = np.zeros((H, TW), np.float32)
    gpad[:, :3969] = gtbl
    gpad = gpad.astype(ml_dtypes.bfloat16)

    bf = ml_dtypes.bfloat16
    wqkvT = np.ascontiguousarray(qkv_w.T.astype(bf))            # [768, 2304]
    wprojT = np.ascontiguousarray(proj_w.T.astype(bf))          # [768, 768]
    pbc = np.ascontiguousarray(
        proj_b.astype(np.float32).reshape(CT, 128).T)           # [128, 6]
    shared = {"qkv_wT": wqkvT, "proj_wT": wprojT, "proj_b_col": pbc,
              "rpb_tbl": gpad}
    in_maps = []
    for i in range(B):
        m = dict(shared)
        m["xT"] = np.ascontiguousarray(x[i].T.astype(bf))
        m["xRT"] = np.ascontiguousarray(x[i][::-1].T.astype(bf))
        in_maps.append(m)
    return in_maps


def kernel(x, qkv_w, proj_w, proj_b, rpb_w1, rpb_b1, rpb_w2, rpb_b2,
           _trace=False, _tmpdir=None):
    in_maps = _host_prep(np.asarray(x), np.asarray(qkv_w), np.asarray(proj_w),
                         np.asarray(proj_b), np.asarray(rpb_w1),
                         np.asarray(rpb_b1), np.asarray(rpb_w2),
                         np.asarray(rpb_b2))
    nc = _graph()
    res = run_bass_kernel_spmd(nc, in_maps, core_ids=list(range(B)),
                               trace=_trace, tmpdir=_tmpdir)
    out = np.stack([np.ascontiguousarray(res.results[i]["out"].T)
                    for i in range(B)]).astype(np.float32)
    if _trace:
        kernel._last_results = res
    return out
